# revision 8
# baseline (speedup 1.0000x reference)
"""Trainium2 Bass kernel for ContextQueryAttn (BiDAF-style trilinear attention).

Computes, per batch b:
    sim = sc[:,None] + sq[None,:] + (ctx*wm) @ query.T          (Lc, Lq)
    sim = where(cmask[:,None] | qmask[None,:], -1e30, sim)
    S   = softmax(sim, axis=-1)   (row softmax over Lq)
    SS  = softmax(sim, axis=0)    (col softmax over Lc)
    A   = S @ query               (Lc, D)
    T   = SS.T @ ctx              (Lq, D)
    B   = S @ T                   (Lc, D)
returns (A, B).

Strategy: data-parallel over batch B=32 across 8 cores (4 batches/core).
bf16 matmul operands, f32 PSUM accumulation, bf16 outputs upcast on host.
ACT/DVE cost ~0.4-0.7us per instruction nearly independent of width and a
DMA ring streams ~360GB/s FIFO per issuing engine, so the design minimizes
instruction counts and spreads DMAs across the sync/scalar/gpsimd rings:
  - inputs ship as two host-packed bf16 blobs per batch (one DMA each, on
    different rings) in SBUF-native layout; 128 x ~9KB lines per DMA.
  - chip ships UNNORMALIZED A_raw (with a ones-column giving rowsum) and
    B_raw; host divides by rowsum.
  - cmasked rows of A/B (uniform softmax rows in the reference) are fixed
    on the host from query.mean and Tn.mean; no on-chip mask override.
  - col-path numerators need no per-row bias: T = colnorm(E_col) is
    invariant to per-q column scaling, so Pc = exp(dot) with the e^{sc[c]}
    scaling (and cmask zeroing) folded into the host-scaled ctx' rhs;
    qmasked columns are repaired by the qf blend with ctxsum.
  - col dots and B matmuls are paired (two 256-wide ci outputs per 512-wide
    PSUM bank), halving exp/drain counts there.
  - software pipelining: B loop of batch b runs inside batch b+1's row
    phase; T/A matmuls skew one ci-pair behind the dot matmuls; the last
    batch runs col-first (T finalize as early as possible), then row, then
    an interleaved A+B loop with split stores, shrinking the serial tail.
Masked-softmax exactness: no max subtraction (logits O(+-10)); qmask folds
as -1e30 into the row-exp bias so exp=0 exactly; cmask rows excluded from
the col softmax by ctx' = 0; fully-masked T rows replaced via q_scale/qf
blend with ctxsum.
"""

import numpy as np
import ml_dtypes

import concourse.bass as bass
import concourse.tile as tile
from concourse import bacc, mybir
from concourse.bass_utils import run_bass_kernel_spmd

F32 = mybir.dt.float32
BF16 = mybir.dt.bfloat16
EXP = mybir.ActivationFunctionType.Exp
ALU = mybir.AluOpType
NPBF = ml_dtypes.bfloat16

B, LC, LQ, D = 32, 2048, 256, 256
NCORES = 8
BPC = B // NCORES          # batches per core
NCT = LC // 128            # 16 context tiles
NQT = LQ // 128            # 2 query tiles
NKD = D // 128             # 2 contraction chunks over D
NCH = LC // 512            # 4 row-path column chunks
NP = NCT // 2              # 8 ci pairs
NEG = np.float32(-1e30)
VW = 6                     # vecs: sqb[0:2], qsc[2:4], qf[4:6]

# blob1: qwmT [NKD,256] then ctxT [NKD,2048]
OFF_QWM = 0
OFF_CTXT = 512
W1 = 512 + NKD * LC        # 4608
# blob2: qext [NQT,258] then ctx' [NCT,258] then csum [258]
OFF_Q = 0
OFF_CTX = NQT * 258        # 516
OFF_CS = OFF_CTX + NCT * 258   # 4644
W2 = OFF_CS + 258          # 4902


def _build_kernel(tc, nc, ins, outs):
    import contextlib
    ctx = contextlib.ExitStack()

    sb = lambda name, bufs: ctx.enter_context(
        tc.tile_pool(name=name, bufs=bufs))
    ps_pool = ctx.enter_context(tc.tile_pool(name="ps", bufs=6, space="PSUM"))
    t_pool = ctx.enter_context(tc.tile_pool(name="tps", bufs=1, space="PSUM"))

    p_b1 = sb("pb1", 2)
    p_b2 = sb("pb2", 2)
    p_PT = sb("pPT", 2)
    p_Pc = sb("pPc", 2)
    p_Tn = sb("pTn", 2)
    p_vec = sb("pvec", 2)
    p_ast = sb("past", 2)
    p_bst = sb("pbst", 2)

    def b_pair(pPT, pTn, B_st, p):
        b_ps = ps_pool.tile([128, 2, 256], F32, tag="ps", name="b_ps")
        for h in range(2):
            for qt in range(NQT):
                nc.tensor.matmul(
                    b_ps[:, h, :],
                    lhsT=pPT[:, qt, bass.ts(2 * p + h, 128)],
                    rhs=pTn[:, qt, :],
                    start=(qt == 0), stop=(qt == NQT - 1))
        if p % 2 == 0:
            nc.scalar.copy(B_st[:, 2 * p:2 * p + 2, :], b_ps[:])
        else:
            nc.vector.tensor_scalar_add(B_st[:, 2 * p:2 * p + 2, :],
                                        b_ps[:], 0.0)

    # ---- pipelined B loop of previous batch ----
    def b_loop(pPT, pTn, pb):
        B_st = p_bst.tile([128, NCT, 256], BF16, name="B_st")
        for p in range(NP):
            b_pair(pPT, pTn, B_st, p)
        nc.gpsimd.dma_start(
            out=outs["Bm"][pb].rearrange("(t p) x -> p t x", p=128),
            in_=B_st[:])

    prev = None

    for b in range(BPC):
        last = (b == BPC - 1)
        # ---- loads: vecs+blob1 on the sync ring, blob2 on the scalar ring
        vec_sb = p_vec.tile([128, VW], F32, name="vec_sb")
        nc.sync.dma_start(out=vec_sb[:], in_=ins["vecs"][b])
        b1 = p_b1.tile([128, W1], BF16, name="b1")
        nc.sync.dma_start(out=b1[:], in_=ins["blob1"][b])
        b2 = p_b2.tile([128, W2], BF16, name="b2")
        nc.scalar.dma_start(out=b2[:], in_=ins["blob2"][b])

        qwmT = lambda kd, off, ln: b1[:, OFF_QWM + kd * 256 + off:
                                      OFF_QWM + kd * 256 + off + ln]
        ctxT = lambda kd, off, ln: b1[:, OFF_CTXT + kd * LC + off:
                                      OFF_CTXT + kd * LC + off + ln]
        qx = lambda qt, ln: b2[:, OFF_Q + qt * 258:OFF_Q + qt * 258 + ln]
        cx = lambda ci: b2[:, OFF_CTX + ci * 258:OFF_CTX + (ci + 1) * 258]
        cs_sb = b2[:, OFF_CS:OFF_CS + 258]

        sqb = lambda qt: vec_sb[:, 0 + qt:1 + qt]
        qsc = lambda qt: vec_sb[:, 2 + qt:3 + qt]
        qfv = lambda qt: vec_sb[:, 4 + qt:5 + qt]

        csrec_sb = p_vec.tile([128, NQT], F32, name="csrec_sb")

        PT_sb = p_PT.tile([128, NQT, LC], BF16, name="PT_sb")
        Pc_sb = p_Pc.tile([128, NCT, LQ], BF16, name="Pc_sb")
        A_st = p_ast.tile([128, NCT, 258], BF16, name="A_st")
        T_ps = [t_pool.tile([128, 258], F32, name=f"T_ps{qt}")
                for qt in range(NQT)]
        Tn_sb = p_Tn.tile([128, NQT, 256], BF16, name="Tn_sb")

        # ---- row path: dotT (q, c) -> exp(+sq_bias) -> P^T (= E_row^T)
        def row_path():
            for ch in range(NCH):
                for qt in range(NQT):
                    dt_ps = ps_pool.tile([128, 512], F32, tag="ps",
                                         name="dt_ps")
                    for kd in range(NKD):
                        nc.tensor.matmul(
                            dt_ps[:],
                            lhsT=qwmT(kd, qt * 128, 128),
                            rhs=ctxT(kd, ch * 512, 512),
                            start=(kd == 0), stop=(kd == NKD - 1))
                    nc.scalar.activation(
                        PT_sb[:, qt, bass.ts(ch, 512)], dt_ps[:], EXP,
                        bias=sqb(qt))

        # ---- col dot pair: -> exp (no bias) -> Pc
        def dot_p(p):
            dps = ps_pool.tile([128, 2, 256], F32, tag="ps", name="dps")
            for h in range(2):
                for kd in range(NKD):
                    nc.tensor.matmul(
                        dps[:, h, :],
                        lhsT=ctxT(kd, (2 * p + h) * 128, 128),
                        rhs=qwmT(kd, 0, 256),
                        start=(kd == 0), stop=(kd == NKD - 1))
            nc.scalar.activation(
                Pc_sb[:, 2 * p:2 * p + 2, :], dps[:], EXP)

        def t_p(p):
            for h in range(2):
                ci = 2 * p + h
                for qt in range(NQT):
                    nc.tensor.matmul(
                        T_ps[qt][:],
                        lhsT=Pc_sb[:, ci, bass.ts(qt, 128)],
                        rhs=cx(ci),
                        start=(ci == 0), stop=(ci == NCT - 1))

        def drain_a(ci, a_ps):
            if ci % 4 == 3:
                nc.scalar.copy(A_st[:, ci, :], a_ps[:])
            else:
                nc.vector.tensor_scalar_add(A_st[:, ci, :], a_ps[:], 0.0)

        def a_ci(ci):
            a_ps = ps_pool.tile([128, 258], F32, tag="ps", name="a_ps")
            for qt in range(NQT):
                nc.tensor.matmul(
                    a_ps[:],
                    lhsT=PT_sb[:, qt, bass.ts(ci, 128)],
                    rhs=qx(qt, 258),
                    start=(qt == 0), stop=(qt == NQT - 1))
            drain_a(ci, a_ps)

        def t_finalize():
            for qt in range(NQT):
                nc.vector.tensor_scalar_mul(T_ps[qt][:], T_ps[qt][:],
                                            qsc(qt))
                nc.vector.scalar_tensor_tensor(
                    out=T_ps[qt][:], in0=cs_sb, scalar=qfv(qt),
                    in1=T_ps[qt][:], op0=ALU.mult, op1=ALU.add)
                nc.vector.reciprocal(csrec_sb[:, qt:qt + 1],
                                     T_ps[qt][:, 256:257])
                nc.scalar.mul(Tn_sb[:, qt, :], T_ps[qt][:, 0:256],
                              csrec_sb[:, qt:qt + 1])
            nc.gpsimd.dma_start(out=outs["Tn"][b], in_=Tn_sb[:])

        if not last:
            row_path()
            if prev is not None:
                b_loop(*prev)
            # col path; T/A skew one pair behind the dots
            dot_p(0)
            for p in range(1, NP):
                dot_p(p)
                t_p(p - 1)
                a_ci(2 * (p - 1))
                a_ci(2 * (p - 1) + 1)
            t_p(NP - 1)
            a_ci(NCT - 2)
            a_ci(NCT - 1)
            nc.sync.dma_start(
                out=outs["A"][b].rearrange("(t p) x -> p t x", p=128),
                in_=A_st[:])
            t_finalize()
            prev = (PT_sb, Tn_sb, b)
        else:
            # last batch: col-first so Tn is ready before the A+B loop
            dot_p(0)
            for p in range(1, NP):
                dot_p(p)
                t_p(p - 1)
            t_p(NP - 1)
            t_finalize()
            row_path()
            b_loop(*prev)
            B_st = p_bst.tile([128, NCT, 256], BF16, name="B_st")
            for p in range(NP):
                a_ci(2 * p)
                a_ci(2 * p + 1)
                b_pair(PT_sb, Tn_sb, B_st, p)
                if p == NP // 2 - 1:
                    nc.sync.dma_start(
                        out=outs["A"][b, 0:LC // 2, :].rearrange(
                            "(t p) x -> p t x", p=128),
                        in_=A_st[:, 0:NCT // 2, :])
                    nc.gpsimd.dma_start(
                        out=outs["Bm"][b, 0:LC // 2, :].rearrange(
                            "(t p) x -> p t x", p=128),
                        in_=B_st[:, 0:NCT // 2, :])
            nc.sync.dma_start(
                out=outs["A"][b, LC // 2:LC, :].rearrange(
                    "(t p) x -> p t x", p=128),
                in_=A_st[:, NCT // 2:NCT, :])
            nc.gpsimd.dma_start(
                out=outs["Bm"][b, LC // 2:LC, :].rearrange(
                    "(t p) x -> p t x", p=128),
                in_=B_st[:, NCT // 2:NCT, :])

    ctx.close()


def build_program():
    nc = bacc.Bacc("TRN2", target_bir_lowering=False, debug=False,
                   num_devices=NCORES)
    ins = {
        "blob1": nc.dram_tensor("blob1", [BPC, 128, W1], BF16,
                                kind="ExternalInput").ap(),
        "blob2": nc.dram_tensor("blob2", [BPC, 128, W2], BF16,
                                kind="ExternalInput").ap(),
        "vecs": nc.dram_tensor("vecs", [BPC, 128, VW], F32,
                               kind="ExternalInput").ap(),
    }
    outs = {
        "A": nc.dram_tensor("A", [BPC, LC, 258], BF16,
                            kind="ExternalOutput").ap(),
        "Bm": nc.dram_tensor("Bm", [BPC, LC, D], BF16,
                             kind="ExternalOutput").ap(),
        "Tn": nc.dram_tensor("Tn", [BPC, 128, NQT, 256], BF16,
                             kind="ExternalOutput").ap(),
    }
    with tile.TileContext(nc) as tc:
        _build_kernel(tc, nc, ins, outs)
    nc.compile()
    return nc


def host_prep(context, query, context_mask, query_mask, w0):
    """Host-side preprocessing: shard + pack SBUF-native bf16 blobs."""
    f = np.float32
    context = np.ascontiguousarray(context, dtype=f)
    query = np.ascontiguousarray(query, dtype=f)
    w0 = np.asarray(w0, dtype=f)
    wc, wq, wm = w0[:D], w0[D:2 * D], w0[2 * D:]
    cf = context_mask.astype(f)
    qf = query_mask.astype(f)
    sc = context @ wc                      # (B, LC)
    sq = query @ wq                        # (B, LQ)

    # blob1 [B, 128, W1] = [qwmT | ctxT]
    qwm = query * wm
    qwmT_p = qwm.reshape(B, LQ, NKD, 128).transpose(0, 3, 2, 1)
    ctxT_p = context.reshape(B, LC, NKD, 128).transpose(0, 3, 2, 1)
    blob1 = np.concatenate(
        [qwmT_p.reshape(B, 128, 512), ctxT_p.reshape(B, 128, NKD * LC)],
        -1).astype(NPBF)

    # blob2 [B, 128, W2] = [qext | ctx' | csum]
    ones_q = np.ones((B, LQ, 1), f)
    zq = np.zeros((B, LQ, 1), f)
    q_ext = np.concatenate([query, ones_q, zq], -1)
    qext_p = q_ext.reshape(B, NQT, 128, 258).transpose(0, 2, 1, 3)
    # ctx' = e^{sc[c]} * (1-cf[c]) * [ctx | 1 | 0]
    esc = (np.exp(sc) * (1.0 - cf))[:, :, None].astype(f)
    ones_c = np.ones((B, LC, 1), f)
    zc = np.zeros((B, LC, 1), f)
    ctx_ext = np.concatenate([context, ones_c, zc], -1) * esc
    ctx_p = ctx_ext.reshape(B, NCT, 128, 258).transpose(0, 2, 1, 3)
    ctxsum = np.concatenate(
        [context.sum(1, dtype=f), np.full((B, 1), LC, f),
         np.zeros((B, 1), f)], -1)
    csum_p = np.broadcast_to(ctxsum[:, None, :], (B, 128, 258))
    blob2 = np.concatenate(
        [qext_p.reshape(B, 128, NQT * 258), ctx_p.reshape(B, 128, NCT * 258),
         csum_p], -1).astype(NPBF)

    # vecs [B, 128, 6]: sqb(2) qsc(2) qf(2); val[p,t] = v[t*128+p]
    q_scale = (1.0 - qf).astype(f)
    sq_bias = (q_scale * sq + qf * NEG).astype(f)
    pq = lambda v: v.reshape(B, NQT, 128).transpose(0, 2, 1)
    vecs = np.ascontiguousarray(np.concatenate(
        [pq(sq_bias), pq(q_scale), pq(qf)], -1))

    full = {"blob1": blob1, "blob2": blob2, "vecs": vecs}
    in_maps = []
    for c in range(NCORES):
        sl = slice(c * BPC, (c + 1) * BPC)
        m = {k: np.ascontiguousarray(v[sl]) for k, v in full.items()}
        in_maps.append(m)
    return in_maps


_cached_nc = None


def get_program():
    global _cached_nc
    if _cached_nc is None:
        _cached_nc = build_program()
    return _cached_nc


def run_on_hw(in_maps, **kwargs):
    nc = get_program()
    return run_bass_kernel_spmd(nc, in_maps, core_ids=list(range(NCORES)),
                                **kwargs)


def host_post(res, context_mask, query):
    """Normalize by rowsum; overwrite cmasked rows (uniform softmax rows)."""
    f = np.float32
    A_ext = np.concatenate(
        [np.asarray(res.results[c]["A"]).astype(f) for c in range(NCORES)], 0)
    A_raw = A_ext[:, :, 0:256]
    rs = A_ext[:, :, 256]
    B_raw = np.concatenate(
        [np.asarray(res.results[c]["Bm"]).astype(f) for c in range(NCORES)], 0)
    TnD = np.concatenate(
        [np.asarray(res.results[c]["Tn"]).astype(f) for c in range(NCORES)], 0)
    # Tn_full[b, t*128+p, :] = TnD[b, p, t, :]
    Tn_full = TnD.transpose(0, 2, 1, 3).reshape(B, LQ, D)

    cm = np.asarray(context_mask, bool)[:, :, None]
    rs_safe = np.where(cm[:, :, 0], f(1.0), rs)[:, :, None]
    qmean = np.asarray(query, f).mean(1)[:, None, :]
    tmean = Tn_full.mean(1)[:, None, :]
    A = np.where(cm, qmean, A_raw / rs_safe)
    Bm = np.where(cm, tmean, B_raw / rs_safe)
    return A, Bm


def kernel(context, query, context_mask, query_mask, w0):
    in_maps = host_prep(context, query, context_mask, query_mask, w0)
    res = run_on_hw(in_maps)
    return host_post(res, context_mask, query)


# revision 9
# speedup vs baseline: 1.1090x; 1.1090x over previous
"""Trainium2 Bass kernel for ContextQueryAttn (BiDAF-style trilinear attention).

Computes, per batch b:
    sim = sc[:,None] + sq[None,:] + (ctx*wm) @ query.T          (Lc, Lq)
    sim = where(cmask[:,None] | qmask[None,:], -1e30, sim)
    S   = softmax(sim, axis=-1)   (row softmax over Lq)
    SS  = softmax(sim, axis=0)    (col softmax over Lc)
    A   = S @ query               (Lc, D)
    T   = SS.T @ ctx              (Lq, D)
    B   = S @ T                   (Lc, D)
returns (A, B).

Strategy: data-parallel over batch B=32 across 8 cores (4 batches/core).
bf16 matmul operands, f32 PSUM accumulation, bf16 outputs upcast on host.
ACT/DVE cost ~0.4-0.7us per instruction nearly independent of width and a
DMA ring streams ~360GB/s FIFO per issuing engine, so the design minimizes
instruction counts and spreads DMAs across the sync/scalar/gpsimd rings:
  - inputs ship as two host-packed bf16 blobs per batch (one DMA each, on
    different rings) in SBUF-native layout; 128 x ~9KB lines per DMA.
  - chip ships UNNORMALIZED A_raw (with a ones-column giving rowsum) and
    B_raw; host divides by rowsum.
  - cmasked rows of A/B (uniform softmax rows in the reference) are fixed
    on the host from query.mean and Tn.mean; no on-chip mask override.
  - col-path numerators need no per-row bias: T = colnorm(E_col) is
    invariant to per-q column scaling, so Pc = exp(dot) with the e^{sc[c]}
    scaling (and cmask zeroing) folded into the host-scaled ctx' rhs;
    qmasked columns are repaired by the qf blend with ctxsum.
  - col dots and B matmuls are paired (two 256-wide ci outputs per 512-wide
    PSUM bank), halving exp/drain counts there.
  - software pipelining: B loop of batch b runs inside batch b+1's row
    phase; T/A matmuls skew one ci-pair behind the dot matmuls; the last
    batch runs col-first (T finalize as early as possible), then row, then
    an interleaved A+B loop with split stores, shrinking the serial tail.
Masked-softmax exactness: no max subtraction (logits O(+-10)); qmask folds
as -1e30 into the row-exp bias so exp=0 exactly; cmask rows excluded from
the col softmax by ctx' = 0; fully-masked T rows replaced via q_scale/qf
blend with ctxsum.
"""

import numpy as np
import ml_dtypes

import concourse.bass as bass
import concourse.tile as tile
from concourse import bacc, mybir
from concourse.bass_utils import run_bass_kernel_spmd

F32 = mybir.dt.float32
BF16 = mybir.dt.bfloat16
EXP = mybir.ActivationFunctionType.Exp
ALU = mybir.AluOpType
NPBF = ml_dtypes.bfloat16

B, LC, LQ, D = 32, 2048, 256, 256
NCORES = 8
BPC = B // NCORES          # batches per core
NCT = LC // 128            # 16 context tiles
NQT = LQ // 128            # 2 query tiles
NKD = D // 128             # 2 contraction chunks over D
NCH = LC // 512            # 4 row-path column chunks
NP = NCT // 2              # 8 ci pairs
NEG = np.float32(-1e30)
VW = 6                     # vecs: sqb[0:2], qsc[2:4], qf[4:6]

# blob1: qwmT [NKD,256] then ctxT [NKD,2048]
OFF_QWM = 0
OFF_CTXT = 512
W1 = 512 + NKD * LC        # 4608
# blob2: qext [NQT,258] then ctx' [NCT,258] then csum [258]
OFF_Q = 0
OFF_CTX = NQT * 258        # 516
OFF_CS = OFF_CTX + NCT * 258   # 4644
W2 = OFF_CS + 258          # 4902


def _build_kernel(tc, nc, ins, outs):
    import contextlib
    ctx = contextlib.ExitStack()

    sb = lambda name, bufs: ctx.enter_context(
        tc.tile_pool(name=name, bufs=bufs))
    ps_pool = ctx.enter_context(tc.tile_pool(name="ps", bufs=6, space="PSUM"))
    t_pool = ctx.enter_context(tc.tile_pool(name="tps", bufs=1, space="PSUM"))

    p_b1 = sb("pb1", 2)
    p_b2 = sb("pb2", 2)
    p_PT = sb("pPT", 2)
    p_Pc = sb("pPc", 2)
    p_Tn = sb("pTn", 2)
    p_vec = sb("pvec", 2)
    p_ast = sb("past", 2)
    p_bst = sb("pbst", 2)

    def b_pair(pPT, pTn, B_st, p):
        b_ps = ps_pool.tile([128, 2, 256], F32, tag="ps", name="b_ps")
        for h in range(2):
            for qt in range(NQT):
                nc.tensor.matmul(
                    b_ps[:, h, :],
                    lhsT=pPT[:, qt, bass.ts(2 * p + h, 128)],
                    rhs=pTn[:, qt, :],
                    start=(qt == 0), stop=(qt == NQT - 1))
        if p % 2 == 0:
            nc.scalar.copy(B_st[:, 2 * p:2 * p + 2, :], b_ps[:])
        else:
            nc.vector.tensor_scalar_add(B_st[:, 2 * p:2 * p + 2, :],
                                        b_ps[:], 0.0)

    # ---- pipelined B loop of previous batch ----
    def b_loop(pPT, pTn, pb):
        B_st = p_bst.tile([128, NCT, 256], BF16, name="B_st")
        for p in range(NP):
            b_pair(pPT, pTn, B_st, p)
        nc.sync.dma_start(
            out=outs["Bm"][pb].rearrange("(t p) x -> p t x", p=128),
            in_=B_st[:])

    prev = None

    for b in range(BPC):
        last = (b == BPC - 1)
        # ---- loads: vecs+blob1 on the sync ring, blob2 on the scalar ring
        vec_sb = p_vec.tile([128, VW], F32, name="vec_sb")
        nc.sync.dma_start(out=vec_sb[:], in_=ins["vecs"][b])
        b1 = p_b1.tile([128, W1], BF16, name="b1")
        nc.sync.dma_start(out=b1[:], in_=ins["blob1"][b])
        b2 = p_b2.tile([128, W2], BF16, name="b2")
        nc.sync.dma_start(out=b2[:], in_=ins["blob2"][b])

        qwmT = lambda kd, off, ln: b1[:, OFF_QWM + kd * 256 + off:
                                      OFF_QWM + kd * 256 + off + ln]
        ctxT = lambda kd, off, ln: b1[:, OFF_CTXT + kd * LC + off:
                                      OFF_CTXT + kd * LC + off + ln]
        qx = lambda qt, ln: b2[:, OFF_Q + qt * 258:OFF_Q + qt * 258 + ln]
        cx = lambda ci: b2[:, OFF_CTX + ci * 258:OFF_CTX + (ci + 1) * 258]
        cs_sb = b2[:, OFF_CS:OFF_CS + 258]

        sqb = lambda qt: vec_sb[:, 0 + qt:1 + qt]
        qsc = lambda qt: vec_sb[:, 2 + qt:3 + qt]
        qfv = lambda qt: vec_sb[:, 4 + qt:5 + qt]

        csrec_sb = p_vec.tile([128, NQT], F32, name="csrec_sb")

        PT_sb = p_PT.tile([128, NQT, LC], BF16, name="PT_sb")
        Pc_sb = p_Pc.tile([128, NCT, LQ], BF16, name="Pc_sb")
        A_st = p_ast.tile([128, NCT, 258], BF16, name="A_st")
        T_ps = [t_pool.tile([128, 258], F32, name=f"T_ps{qt}")
                for qt in range(NQT)]
        Tn_sb = p_Tn.tile([128, NQT, 256], BF16, name="Tn_sb")

        # ---- row path: dotT (q, c) -> exp(+sq_bias) -> P^T (= E_row^T)
        def row_path():
            for ch in range(NCH):
                for qt in range(NQT):
                    dt_ps = ps_pool.tile([128, 512], F32, tag="ps",
                                         name="dt_ps")
                    for kd in range(NKD):
                        nc.tensor.matmul(
                            dt_ps[:],
                            lhsT=qwmT(kd, qt * 128, 128),
                            rhs=ctxT(kd, ch * 512, 512),
                            start=(kd == 0), stop=(kd == NKD - 1))
                    nc.scalar.activation(
                        PT_sb[:, qt, bass.ts(ch, 512)], dt_ps[:], EXP,
                        bias=sqb(qt))

        # ---- col dot pair: -> exp (no bias) -> Pc
        def dot_p(p):
            dps = ps_pool.tile([128, 2, 256], F32, tag="ps", name="dps")
            for h in range(2):
                for kd in range(NKD):
                    nc.tensor.matmul(
                        dps[:, h, :],
                        lhsT=ctxT(kd, (2 * p + h) * 128, 128),
                        rhs=qwmT(kd, 0, 256),
                        start=(kd == 0), stop=(kd == NKD - 1))
            nc.scalar.activation(
                Pc_sb[:, 2 * p:2 * p + 2, :], dps[:], EXP)

        def t_p(p):
            for h in range(2):
                ci = 2 * p + h
                for qt in range(NQT):
                    nc.tensor.matmul(
                        T_ps[qt][:],
                        lhsT=Pc_sb[:, ci, bass.ts(qt, 128)],
                        rhs=cx(ci),
                        start=(ci == 0), stop=(ci == NCT - 1))

        def drain_a(ci, a_ps):
            if ci % 4 == 3:
                nc.scalar.copy(A_st[:, ci, :], a_ps[:])
            else:
                nc.vector.tensor_scalar_add(A_st[:, ci, :], a_ps[:], 0.0)

        def a_ci(ci):
            a_ps = ps_pool.tile([128, 258], F32, tag="ps", name="a_ps")
            for qt in range(NQT):
                nc.tensor.matmul(
                    a_ps[:],
                    lhsT=PT_sb[:, qt, bass.ts(ci, 128)],
                    rhs=qx(qt, 258),
                    start=(qt == 0), stop=(qt == NQT - 1))
            drain_a(ci, a_ps)

        def t_finalize():
            for qt in range(NQT):
                nc.vector.tensor_scalar_mul(T_ps[qt][:], T_ps[qt][:],
                                            qsc(qt))
                nc.vector.scalar_tensor_tensor(
                    out=T_ps[qt][:], in0=cs_sb, scalar=qfv(qt),
                    in1=T_ps[qt][:], op0=ALU.mult, op1=ALU.add)
                nc.vector.reciprocal(csrec_sb[:, qt:qt + 1],
                                     T_ps[qt][:, 256:257])
                nc.scalar.mul(Tn_sb[:, qt, :], T_ps[qt][:, 0:256],
                              csrec_sb[:, qt:qt + 1])
            nc.sync.dma_start(out=outs["Tn"][b], in_=Tn_sb[:])

        if not last:
            row_path()
            if prev is not None:
                b_loop(*prev)
            # col path; T/A skew one pair behind the dots
            dot_p(0)
            for p in range(1, NP):
                dot_p(p)
                t_p(p - 1)
                a_ci(2 * (p - 1))
                a_ci(2 * (p - 1) + 1)
            t_p(NP - 1)
            a_ci(NCT - 2)
            a_ci(NCT - 1)
            nc.sync.dma_start(
                out=outs["A"][b].rearrange("(t p) x -> p t x", p=128),
                in_=A_st[:])
            t_finalize()
            prev = (PT_sb, Tn_sb, b)
        else:
            # last batch: col-first so Tn is ready before the A+B loop
            dot_p(0)
            for p in range(1, NP):
                dot_p(p)
                t_p(p - 1)
            t_p(NP - 1)
            t_finalize()
            row_path()
            b_loop(*prev)
            B_st = p_bst.tile([128, NCT, 256], BF16, name="B_st")
            for p in range(NP):
                a_ci(2 * p)
                a_ci(2 * p + 1)
                b_pair(PT_sb, Tn_sb, B_st, p)
                if p == NP // 2 - 1:
                    nc.sync.dma_start(
                        out=outs["A"][b, 0:LC // 2, :].rearrange(
                            "(t p) x -> p t x", p=128),
                        in_=A_st[:, 0:NCT // 2, :])
                    nc.sync.dma_start(
                        out=outs["Bm"][b, 0:LC // 2, :].rearrange(
                            "(t p) x -> p t x", p=128),
                        in_=B_st[:, 0:NCT // 2, :])
            nc.sync.dma_start(
                out=outs["A"][b, LC // 2:LC, :].rearrange(
                    "(t p) x -> p t x", p=128),
                in_=A_st[:, NCT // 2:NCT, :])
            nc.sync.dma_start(
                out=outs["Bm"][b, LC // 2:LC, :].rearrange(
                    "(t p) x -> p t x", p=128),
                in_=B_st[:, NCT // 2:NCT, :])

    ctx.close()


def build_program():
    nc = bacc.Bacc("TRN2", target_bir_lowering=False, debug=False,
                   num_devices=NCORES)
    ins = {
        "blob1": nc.dram_tensor("blob1", [BPC, 128, W1], BF16,
                                kind="ExternalInput").ap(),
        "blob2": nc.dram_tensor("blob2", [BPC, 128, W2], BF16,
                                kind="ExternalInput").ap(),
        "vecs": nc.dram_tensor("vecs", [BPC, 128, VW], F32,
                               kind="ExternalInput").ap(),
    }
    outs = {
        "A": nc.dram_tensor("A", [BPC, LC, 258], BF16,
                            kind="ExternalOutput").ap(),
        "Bm": nc.dram_tensor("Bm", [BPC, LC, D], BF16,
                             kind="ExternalOutput").ap(),
        "Tn": nc.dram_tensor("Tn", [BPC, 128, NQT, 256], BF16,
                             kind="ExternalOutput").ap(),
    }
    with tile.TileContext(nc) as tc:
        _build_kernel(tc, nc, ins, outs)
    nc.compile()
    return nc


def host_prep(context, query, context_mask, query_mask, w0):
    """Host-side preprocessing: shard + pack SBUF-native bf16 blobs."""
    f = np.float32
    context = np.ascontiguousarray(context, dtype=f)
    query = np.ascontiguousarray(query, dtype=f)
    w0 = np.asarray(w0, dtype=f)
    wc, wq, wm = w0[:D], w0[D:2 * D], w0[2 * D:]
    cf = context_mask.astype(f)
    qf = query_mask.astype(f)
    sc = context @ wc                      # (B, LC)
    sq = query @ wq                        # (B, LQ)

    # blob1 [B, 128, W1] = [qwmT | ctxT]
    qwm = query * wm
    qwmT_p = qwm.reshape(B, LQ, NKD, 128).transpose(0, 3, 2, 1)
    ctxT_p = context.reshape(B, LC, NKD, 128).transpose(0, 3, 2, 1)
    blob1 = np.concatenate(
        [qwmT_p.reshape(B, 128, 512), ctxT_p.reshape(B, 128, NKD * LC)],
        -1).astype(NPBF)

    # blob2 [B, 128, W2] = [qext | ctx' | csum]
    ones_q = np.ones((B, LQ, 1), f)
    zq = np.zeros((B, LQ, 1), f)
    q_ext = np.concatenate([query, ones_q, zq], -1)
    qext_p = q_ext.reshape(B, NQT, 128, 258).transpose(0, 2, 1, 3)
    # ctx' = e^{sc[c]} * (1-cf[c]) * [ctx | 1 | 0]
    esc = (np.exp(sc) * (1.0 - cf))[:, :, None].astype(f)
    ones_c = np.ones((B, LC, 1), f)
    zc = np.zeros((B, LC, 1), f)
    ctx_ext = np.concatenate([context, ones_c, zc], -1) * esc
    ctx_p = ctx_ext.reshape(B, NCT, 128, 258).transpose(0, 2, 1, 3)
    ctxsum = np.concatenate(
        [context.sum(1, dtype=f), np.full((B, 1), LC, f),
         np.zeros((B, 1), f)], -1)
    csum_p = np.broadcast_to(ctxsum[:, None, :], (B, 128, 258))
    blob2 = np.concatenate(
        [qext_p.reshape(B, 128, NQT * 258), ctx_p.reshape(B, 128, NCT * 258),
         csum_p], -1).astype(NPBF)

    # vecs [B, 128, 6]: sqb(2) qsc(2) qf(2); val[p,t] = v[t*128+p]
    q_scale = (1.0 - qf).astype(f)
    sq_bias = (q_scale * sq + qf * NEG).astype(f)
    pq = lambda v: v.reshape(B, NQT, 128).transpose(0, 2, 1)
    vecs = np.ascontiguousarray(np.concatenate(
        [pq(sq_bias), pq(q_scale), pq(qf)], -1))

    full = {"blob1": blob1, "blob2": blob2, "vecs": vecs}
    in_maps = []
    for c in range(NCORES):
        sl = slice(c * BPC, (c + 1) * BPC)
        m = {k: np.ascontiguousarray(v[sl]) for k, v in full.items()}
        in_maps.append(m)
    return in_maps


_cached_nc = None


def get_program():
    global _cached_nc
    if _cached_nc is None:
        _cached_nc = build_program()
    return _cached_nc


def run_on_hw(in_maps, **kwargs):
    nc = get_program()
    return run_bass_kernel_spmd(nc, in_maps, core_ids=list(range(NCORES)),
                                **kwargs)


def host_post(res, context_mask, query):
    """Normalize by rowsum; overwrite cmasked rows (uniform softmax rows)."""
    f = np.float32
    A_ext = np.concatenate(
        [np.asarray(res.results[c]["A"]).astype(f) for c in range(NCORES)], 0)
    A_raw = A_ext[:, :, 0:256]
    rs = A_ext[:, :, 256]
    B_raw = np.concatenate(
        [np.asarray(res.results[c]["Bm"]).astype(f) for c in range(NCORES)], 0)
    TnD = np.concatenate(
        [np.asarray(res.results[c]["Tn"]).astype(f) for c in range(NCORES)], 0)
    # Tn_full[b, t*128+p, :] = TnD[b, p, t, :]
    Tn_full = TnD.transpose(0, 2, 1, 3).reshape(B, LQ, D)

    cm = np.asarray(context_mask, bool)[:, :, None]
    rs_safe = np.where(cm[:, :, 0], f(1.0), rs)[:, :, None]
    qmean = np.asarray(query, f).mean(1)[:, None, :]
    tmean = Tn_full.mean(1)[:, None, :]
    A = np.where(cm, qmean, A_raw / rs_safe)
    Bm = np.where(cm, tmean, B_raw / rs_safe)
    return A, Bm


def kernel(context, query, context_mask, query_mask, w0):
    in_maps = host_prep(context, query, context_mask, query_mask, w0)
    res = run_on_hw(in_maps)
    return host_post(res, context_mask, query)
